# revision 3
# baseline (speedup 1.0000x reference)
"""Llama GQA attention (b=2, s=2048, h=4096, 32 Q heads / 8 KV heads, rope)
as a Bass/Tile kernel for 8 Trainium2 NeuronCores.

Sharding: data-parallel over batch (2) x tensor-parallel over heads (4).
Core c = (b, r), b = c // 4, r = c % 4 handles batch b with Q heads
[8r, 8r+8) and KV heads [2r, 2r+2).  Wq/Wk/Wv column-sharded, Wo
row-sharded; per-core output is a partial sum over the TP group which the
host reduces (fp32 adds).

On-core dataflow (all activations feature-major, i.e. transposed):
  XT [H, T] is DMA'd into SBUF ONCE and stays resident through phase 1;
  all projection matmuls stream it from SBUF (no HBM refetch).
  QT/KT [heads*128, T] with RoPE fused at PSUM eviction; V [T, 256]
  token-major, its transposing matmuls fused into the single-Q-head jobs
  so their ldweights hide under the Q matmul streams.
  Attention runs as a software pipeline over (q-tile, head) jobs:
  S^T[k,q] = KT-tile.T @ QT (PE) + causal mask (DVE) + exp (ACT) of job N
  interleave with AV matmuls of job N-1, so ACT never gates the PE.
  Row sums via DVE accumulation of the exp tiles plus ONE ones-vector
  matmul per job (not per tile); normalization applied at O^T eviction
  through a PE outer-product broadcast of 1/sum.
  O^T tiles feed the Wo projection producing OUT^T [H, T] which the host
  transposes / reduces.
"""

import math
import sys

import numpy as np

for _p in ("/opt/trn_rl_repo",):
    if _p not in sys.path:
        sys.path.insert(0, _p)

import ml_dtypes  # noqa: E402

import concourse.bass as bass  # noqa: E402
import concourse.mybir as mybir  # noqa: E402
import concourse.tile as tile  # noqa: E402
from concourse.alu_op_type import AluOpType  # noqa: E402

F32 = mybir.dt.float32
BF16 = mybir.dt.bfloat16
AF = mybir.ActivationFunctionType

# full problem constants
B, S, H = 2, 2048, 4096
NUM_HEADS, NUM_KV_HEADS, D = 32, 8, 128
ROPE_THETA = 10000.0
TP, DP = 4, 2
MASK_VAL = -30000.0


def build_nc(T=S, HID=H, NQL=NUM_HEADS // TP, NKVL=NUM_KV_HEADS // TP,
             HOUT=H, TQ=512):
    """One-core SPMD program.  T tokens, HID hidden, NQL local Q heads,
    NKVL local KV heads, HOUT output features, TQ q-tile width."""
    assert T % TQ == 0 and TQ % 128 == 0 and HID % 128 == 0
    GRP = NQL // NKVL            # q heads per kv head
    QC = NQL * D                 # local q columns
    KC = NKVL * D                # local kv columns
    KX = HID // 128              # contraction chunks for projections
    NTOK = T // TQ               # token tiles of width TQ
    NT128 = T // 128             # token tiles of width 128
    NKT = TQ // 128              # 128-wide k tiles per q tile
    MW = TQ + (TQ - 128)         # additive causal mask width

    nc = bass.Bass()
    xt = nc.dram_tensor("xt", [HID, T], BF16, kind="ExternalInput")
    wq = nc.dram_tensor("wq", [HID, QC], BF16, kind="ExternalInput")
    wk = nc.dram_tensor("wk", [HID, KC], BF16, kind="ExternalInput")
    wv = nc.dram_tensor("wv", [HID, KC], BF16, kind="ExternalInput")
    wo = nc.dram_tensor("wo", [QC, HOUT], BF16, kind="ExternalInput")
    cosb = nc.dram_tensor("cosb", [128, T], BF16, kind="ExternalInput")
    sinb = nc.dram_tensor("sinb", [128, T], BF16, kind="ExternalInput")  # sign-folded
    maskb = nc.dram_tensor("maskb", [128, MW], BF16, kind="ExternalInput")
    outp = nc.dram_tensor("outp", [HOUT, T], F32, kind="ExternalOutput")

    inv_sqrt_d = 1.0 / math.sqrt(D)

    with tile.TileContext(nc) as tc:
        with (
            tc.tile_pool(name="resident", bufs=1) as res,
            tc.tile_pool(name="const", bufs=1) as const,
        ):
            # resident SBUF arrays (live across all phases)
            qt = [res.tile([128, T], BF16, tag=f"qt{h}", name=f"qt{h}") for h in range(NQL)]
            kt = [res.tile([128, T], BF16, tag=f"kt{h}", name=f"kt{h}") for h in range(NKVL)]
            vt = [res.tile([128, KC], BF16, tag=f"v{t}", name=f"v{t}") for t in range(NT128)]
            cos_sb = res.tile([128, T], BF16, tag="cos", name="cos_sb")
            sin_sb = res.tile([128, T], BF16, tag="sin", name="sin_sb")
            mask_sb = res.tile([128, MW], BF16, tag="mask", name="mask_sb")
            nc.sync.dma_start(out=cos_sb[:], in_=cosb[:])
            nc.sync.dma_start(out=sin_sb[:], in_=sinb[:])
            nc.sync.dma_start(out=mask_sb[:], in_=maskb[:])
            ones_col = const.tile([128, 1], BF16)
            ones_row = const.tile([1, 128], BF16)
            nc.vector.memset(ones_col[:], 1.0)
            nc.vector.memset(ones_row[:], 1.0)

            # ---------------- phase 1: projections ----------------
            # XT resident in SBUF for the whole phase: fetched from HBM once.
            with (
                tc.tile_pool(name="xts", bufs=1) as xpool,
                tc.tile_pool(name="wt", bufs=2) as wpool,
                tc.tile_pool(name="rope_tmp", bufs=2) as rpool,
                tc.tile_pool(name="pj_psum", bufs=1, space="PSUM") as pp,
            ):
                xtsb = [xpool.tile([128, T], BF16, tag=f"x{k}", name=f"x{k}")
                        for k in range(KX)]
                for k in range(KX):
                    nc.sync.dma_start(out=xtsb[k][:],
                                      in_=xt[k * 128:(k + 1) * 128, :])

                def rope_evict(ps, dst_ap, tok0):
                    """dst = ps*cos + rot_half(ps)*sin  (sin sign-folded)."""
                    cw = cos_sb[:, tok0:tok0 + TQ]
                    sw = sin_sb[:, tok0:tok0 + TQ]
                    r = rpool.tile([128, TQ], F32, tag="rot", name="rot")
                    nc.scalar.copy(r[0:64, :], ps[64:128, :])
                    nc.scalar.copy(r[64:128, :], ps[0:64, :])
                    t1 = rpool.tile([128, TQ], F32, tag="t1", name="t1")
                    nc.vector.tensor_tensor(t1[:], ps[:], cw, op=AluOpType.mult)
                    nc.vector.tensor_tensor(r[:], r[:], sw, op=AluOpType.mult)
                    nc.vector.tensor_tensor(dst_ap, t1[:], r[:], op=AluOpType.add)

                # Jobs: (wsrc, dst, h0, nheads, vgroup).  K job first, then Q
                # head-pairs, then Q singles each fused with a V quarter-sweep
                # (V ldweights hide under the Q matmul streams).
                VG = 4
                jobs = [(wk, kt, 0, 2, None),
                        (wq, qt, 0, 2, None),
                        (wq, qt, 2, 2, None)]
                for i in range(4):
                    jobs.append((wq, qt, 4 + i, 1, i * VG))

                for wsrc, dst, h0, nheads, vg0 in jobs:
                    ps = [[pp.tile([128, TQ], F32, tag=f"pp{i * NTOK + j}",
                                   name=f"pj{i}{j}")
                           for j in range(NTOK)] for i in range(nheads)]
                    if vg0 is not None:
                        psv = [pp.tile([128, TQ], F32, tag=f"pp{4 + i}",
                                       name=f"pv{i}")
                               for i in range(VG)]
                    for k in range(KX):
                        wt_sb = wpool.tile([128, nheads * 128], BF16, tag="w", name="wt_sb")
                        nc.sync.dma_start(
                            out=wt_sb[:],
                            in_=wsrc[k * 128:(k + 1) * 128,
                                     h0 * 128:(h0 + nheads) * 128])
                        if vg0 is not None:
                            wv_sb = wpool.tile([128, KC], BF16, tag="wv", name="wv_sb")
                            nc.sync.dma_start(out=wv_sb[:],
                                              in_=wv[k * 128:(k + 1) * 128, :])
                        for i in range(nheads):
                            for j in range(NTOK):
                                nc.tensor.matmul(
                                    ps[i][j][:],
                                    lhsT=wt_sb[:, i * 128:(i + 1) * 128],
                                    rhs=xtsb[k][:, j * TQ:(j + 1) * TQ],
                                    start=(k == 0), stop=(k == KX - 1))
                        if vg0 is not None:
                            for i in range(VG):
                                t0 = (vg0 + i) * 128
                                nc.tensor.matmul(
                                    psv[i][:, 0:KC],
                                    lhsT=xtsb[k][:, t0:t0 + 128],
                                    rhs=wv_sb[:],
                                    start=(k == 0), stop=(k == KX - 1))
                    for i in range(nheads):
                        for j in range(NTOK):
                            rope_evict(ps[i][j], dst[h0 + i][:, j * TQ:(j + 1) * TQ],
                                       j * TQ)
                    if vg0 is not None:
                        for i in range(VG):
                            nc.vector.tensor_copy(vt[vg0 + i][:], psv[i][:, 0:KC])

            # ---------------- phases 2+3 ----------------
            with tc.tile_pool(name="post", bufs=1) as post:
                ot = [post.tile([128, T], BF16, tag=f"ot{h}", name=f"ot{h}")
                      for h in range(NQL)]

                # phase 2: attention, software-pipelined over (qi, h) jobs.
                # Job N's S^T matmuls + mask + exp + DVE row-sum accumulation
                # interleave with job N-1's AV matmuls; one ones-col matmul
                # per job turns the accumulated exp-sum into row sums, and the
                # 1/sum broadcast-normalize runs off the PE's critical path.
                with (
                    tc.tile_pool(name="es", bufs=2) as epool,
                    tc.tile_pool(name="esum", bufs=2) as sumpool,
                    tc.tile_pool(name="at_small", bufs=2) as spool,
                    tc.tile_pool(name="ps_s", bufs=3, space="PSUM") as psum_s,
                    tc.tile_pool(name="ps_o", bufs=2, space="PSUM") as psum_o,
                    tc.tile_pool(name="ps_r", bufs=1, space="PSUM") as psum_r,
                    tc.tile_pool(name="ps_b", bufs=1, space="PSUM") as psum_b,
                ):
                    def emit_s_tile(job, ki):
                        ps_s = psum_s.tile([128, TQ], F32, tag="s", name="ps_s")
                        nc.tensor.matmul(
                            ps_s[:],
                            lhsT=kt[job["kvh"]][:, ki * 128:(ki + 1) * 128],
                            rhs=qt[job["h"]][:, job["q0"]:job["q0"] + TQ],
                            start=True, stop=True)
                        if ki >= job["nk"] - NKT:  # diagonal band: add mask
                            off = ki * 128 - job["q0"]
                            mv = mask_sb[:, (TQ - 128) - off:
                                         (TQ - 128) - off + TQ]
                            nc.vector.tensor_tensor(ps_s[:], ps_s[:], mv,
                                                    op=AluOpType.add)
                        e = epool.tile([128, TQ], BF16, tag=f"e{ki}",
                                       name=f"e{ki}")
                        nc.scalar.activation(e[:], ps_s[:], AF.Exp,
                                             scale=inv_sqrt_d)
                        job["es"].append(e)
                        if ki == 1:
                            esum = sumpool.tile([128, TQ], F32, tag="esum", name="esum")
                            nc.vector.tensor_tensor(
                                esum[:], job["es"][0][:], e[:],
                                op=AluOpType.add)
                            job["esum"] = esum
                        elif ki >= 2:
                            nc.vector.tensor_tensor(
                                job["esum"][:], job["esum"][:], e[:],
                                op=AluOpType.add)

                    def emit_av_tile(job, ki):
                        if ki == 0:
                            job["ps_o"] = psum_o.tile([128, TQ], F32, tag="o", name="ps_o")
                        nc.tensor.matmul(
                            job["ps_o"][:],
                            lhsT=vt[ki][:, job["kvh"] * D:(job["kvh"] + 1) * D],
                            rhs=job["es"][ki][:],
                            start=(ki == 0), stop=(ki == job["nk"] - 1))

                    def emit_norm(job):
                        esb = spool.tile([128, TQ], BF16, tag="esb", name="esb")
                        nc.vector.tensor_copy(esb[:], job["esum"][:])
                        ps_row = psum_r.tile([1, TQ], F32, tag="row", name="ps_row")
                        nc.tensor.matmul(ps_row[:], lhsT=ones_col[:],
                                         rhs=esb[:], start=True, stop=True)
                        rc = spool.tile([1, TQ], F32, tag="rc", name="rc")
                        nc.vector.reciprocal(rc[:], ps_row[:])
                        rcb = spool.tile([1, TQ], BF16, tag="rcb", name="rcb")
                        nc.vector.tensor_copy(rcb[:], rc[:])
                        ps_bc = psum_b.tile([128, TQ], F32, tag="b", name="ps_bc")
                        nc.tensor.matmul(ps_bc[:], lhsT=ones_row[:],
                                         rhs=rcb[:], start=True, stop=True)
                        bc = spool.tile([128, TQ], F32, tag="bc", name="bc")
                        nc.vector.tensor_copy(bc[:], ps_bc[:])
                        nc.vector.tensor_tensor(
                            ot[job["h"]][:, job["q0"]:job["q0"] + TQ],
                            job["ps_o"][:], bc[:], op=AluOpType.mult)

                    prev = None
                    for qi in range(NTOK):
                        for h in range(NQL):
                            cur = {"qi": qi, "h": h, "kvh": h // GRP,
                                   "nk": (qi + 1) * NKT, "q0": qi * TQ,
                                   "es": []}
                            nk_p = prev["nk"] if prev else 0
                            for ki in range(max(cur["nk"], nk_p)):
                                if ki < cur["nk"]:
                                    emit_s_tile(cur, ki)
                                if prev is not None and ki < nk_p:
                                    emit_av_tile(prev, ki)
                            if prev is not None:
                                emit_norm(prev)
                            prev = cur
                    for ki in range(prev["nk"]):
                        emit_av_tile(prev, ki)
                    emit_norm(prev)

                # ---------------- phase 3: output projection ----------------
                CT = QC // 128  # contraction chunks (== NQL)
                with (
                    tc.tile_pool(name="wo_sb", bufs=2) as wopool,
                    tc.tile_pool(name="ob", bufs=4) as obpool,
                    tc.tile_pool(name="po_psum", bufs=2, space="PSUM") as pop,
                ):
                    NG = 4  # n-tiles per weight fetch group
                    for ng in range(0, HOUT // 128, NG):
                        gn = min(NG, HOUT // 128 - ng)
                        wos = []
                        for c in range(CT):
                            w = wopool.tile([128, gn * 128], BF16,
                                            tag=f"wo{c}", name=f"wosb{c}")
                            nc.sync.dma_start(
                                out=w[:], in_=wo[c * 128:(c + 1) * 128,
                                                ng * 128:(ng + gn) * 128])
                            wos.append(w)
                        for i in range(gn):
                            ni = ng + i
                            ps = [pop.tile([128, TQ], F32, tag=f"po{j}",
                                           name=f"po{j}")
                                  for j in range(NTOK)]
                            for c in range(CT):
                                for j in range(NTOK):
                                    nc.tensor.matmul(
                                        ps[j][:],
                                        lhsT=wos[c][:, i * 128:(i + 1) * 128],
                                        rhs=ot[c][:, j * TQ:(j + 1) * TQ],
                                        start=(c == 0), stop=(c == CT - 1))
                            for j in range(NTOK):
                                ob = obpool.tile([128, TQ], F32, tag="ob", name="ob")
                                nc.scalar.copy(ob[:], ps[j][:])
                                nc.sync.dma_start(
                                    out=outp[ni * 128:(ni + 1) * 128,
                                             j * TQ:(j + 1) * TQ],
                                    in_=ob[:])
    legalize_wait_counts(nc)
    return nc


def legalize_wait_counts(nc):
    """walrus DIRECT2D descriptors accept a single sync-wait; Tile can emit
    more (data wait + queue-head wait).  Hoist excess waits onto
    EventSemaphore instructions inserted just before, on the same engine."""
    n_new = 0
    for f in nc.m.functions:
        for blk in f.blocks:
            idx = 0
            insts = blk.instructions
            while idx < len(insts):
                inst = insts[idx]
                si = getattr(inst, "sync_info", None)
                cap = 2 if isinstance(inst, mybir.InstEventSemaphore) else 1
                waits = list(si.on_wait) if si is not None and si.on_wait else []
                if len(waits) > cap:
                    keep, extra = waits[-cap:], waits[:-cap]
                    si.on_wait = keep
                    for i in range(0, len(extra), 2):
                        ev = mybir.InstEventSemaphore(
                            name=f"waitsplit_{n_new}", ins=[], outs=[])
                        n_new += 1
                        ev.engine = inst.engine
                        ev.sync_info = mybir.SyncInfo(
                            on_wait=extra[i:i + 2], on_update=[])
                        nc.register_instruction(ev)
                        insts.insert(idx, ev)
                        idx += 1
                idx += 1
    return n_new


def _host_inputs(hidden_states, position_ids, Wq, Wk, Wv, Wo):
    """Build the 8 per-core input maps."""
    hs = np.asarray(hidden_states, dtype=np.float32)
    pos = np.asarray(position_ids)
    Wq = np.asarray(Wq, dtype=np.float32)
    Wk = np.asarray(Wk, dtype=np.float32)
    Wv = np.asarray(Wv, dtype=np.float32)
    Wo = np.asarray(Wo, dtype=np.float32)
    b, s, h = hs.shape
    qc = h // TP
    kc = (NUM_KV_HEADS * D) // TP
    bf = ml_dtypes.bfloat16

    # rope tables per batch, feature-major, sin sign-folded for rotate_half
    inv_freq = 1.0 / (ROPE_THETA ** (np.arange(0, D, 2, dtype=np.float32) / D))
    maps = []
    TQ = 512
    mw = TQ + (TQ - 128)
    i_idx = np.arange(128)[:, None]
    m_idx = np.arange(mw)[None, :]
    maskb = np.where(m_idx >= i_idx + (TQ - 128), 0.0, MASK_VAL).astype(bf)

    for c in range(DP * TP):
        bb, r = c // TP, c % TP
        t = pos[bb].astype(np.float64)  # [s]
        ang = t[None, :] * np.concatenate([inv_freq, inv_freq])[:, None]  # [128, s]
        cosb = np.cos(ang).astype(bf)
        sinb = np.sin(ang)
        sinb[0:64, :] *= -1.0  # rotate_half sign fold
        sinb = sinb.astype(bf)
        maps.append({
            "xt": np.ascontiguousarray(hs[bb].T).astype(bf),
            "wq": np.ascontiguousarray(Wq[:, r * qc:(r + 1) * qc]).astype(bf),
            "wk": np.ascontiguousarray(Wk[:, r * kc:(r + 1) * kc]).astype(bf),
            "wv": np.ascontiguousarray(Wv[:, r * kc:(r + 1) * kc]).astype(bf),
            "wo": np.ascontiguousarray(Wo[r * qc:(r + 1) * qc, :]).astype(bf),
            "cosb": cosb,
            "sinb": sinb,
            "maskb": maskb,
        })
    return maps


_NC_CACHE = {}


def _get_nc():
    if "nc" not in _NC_CACHE:
        _NC_CACHE["nc"] = build_nc()
    return _NC_CACHE["nc"]


def kernel(hidden_states, position_ids, Wq, Wk, Wv, Wo, _results_hook=None):
    from concourse.bass_utils import run_bass_kernel_spmd

    maps = _host_inputs(hidden_states, position_ids, Wq, Wk, Wv, Wo)
    nc = _get_nc()
    res = run_bass_kernel_spmd(nc, maps, list(range(DP * TP)))
    if _results_hook is not None:
        _results_hook(res)
    b, s, h = np.asarray(hidden_states).shape
    out = np.zeros((b, s, h), dtype=np.float32)
    for c in range(DP * TP):
        bb = c // TP
        out[bb] += res.results[c]["outp"].T
    return out


if __name__ == "__main__":
    # smoke: build the full-size program and print instruction counts
    nc = build_nc()
    print("built ok")


# revision 4
# speedup vs baseline: 1.0402x; 1.0402x over previous
"""Llama GQA attention (b=2, s=2048, h=4096, 32 Q heads / 8 KV heads, rope)
as a Bass/Tile kernel for 8 Trainium2 NeuronCores.

Sharding: data-parallel over batch (2) x tensor-parallel over heads (4).
Core c = (b, r), b = c // 4, r = c % 4 handles batch b with Q heads
[8r, 8r+8) and KV heads [2r, 2r+2).  Wq/Wk/Wv column-sharded, Wo
row-sharded; per-core output is a partial sum over the TP group which the
host reduces (fp32 adds).

On-core dataflow (all activations feature-major, i.e. transposed):
  XT [H, T] is DMA'd into SBUF ONCE and stays resident through phase 1;
  all projection matmuls stream it from SBUF (no HBM refetch).
  QT/KT [heads*128, T] with RoPE fused at PSUM eviction; V [T, 256]
  token-major, its transposing matmuls fused into the single-Q-head jobs
  so their ldweights hide under the Q matmul streams.
  Attention runs as a software pipeline over (q-tile, head) jobs:
  S^T[k,q] = KT-tile.T @ QT (PE) + causal mask (DVE) + exp (ACT) of job N
  interleave with AV matmuls of job N-1, so ACT never gates the PE.
  Row sums via DVE accumulation of the exp tiles plus ONE ones-vector
  matmul per job (not per tile); normalization applied at O^T eviction
  through a PE outer-product broadcast of 1/sum.
  O^T tiles feed the Wo projection producing OUT^T [H, T] which the host
  transposes / reduces.
"""

import math
import sys

import numpy as np

for _p in ("/opt/trn_rl_repo",):
    if _p not in sys.path:
        sys.path.insert(0, _p)

import ml_dtypes  # noqa: E402

import concourse.bass as bass  # noqa: E402
import concourse.mybir as mybir  # noqa: E402
import concourse.tile as tile  # noqa: E402
from concourse.alu_op_type import AluOpType  # noqa: E402

F32 = mybir.dt.float32
BF16 = mybir.dt.bfloat16
AF = mybir.ActivationFunctionType

# full problem constants
B, S, H = 2, 2048, 4096
NUM_HEADS, NUM_KV_HEADS, D = 32, 8, 128
ROPE_THETA = 10000.0
TP, DP = 4, 2
MASK_VAL = -30000.0


def build_nc(T=S, HID=H, NQL=NUM_HEADS // TP, NKVL=NUM_KV_HEADS // TP,
             HOUT=H, TQ=512):
    """One-core SPMD program.  T tokens, HID hidden, NQL local Q heads,
    NKVL local KV heads, HOUT output features, TQ q-tile width."""
    assert T % TQ == 0 and TQ % 128 == 0 and HID % 128 == 0
    GRP = NQL // NKVL            # q heads per kv head
    QC = NQL * D                 # local q columns
    KC = NKVL * D                # local kv columns
    KX = HID // 128              # contraction chunks for projections
    NTOK = T // TQ               # token tiles of width TQ
    NT128 = T // 128             # token tiles of width 128
    NKT = TQ // 128              # 128-wide k tiles per q tile
    MW = TQ + (TQ - 128)         # additive causal mask width

    nc = bass.Bass()
    xt = nc.dram_tensor("xt", [HID, T], BF16, kind="ExternalInput")
    wq = nc.dram_tensor("wq", [HID, QC], BF16, kind="ExternalInput")
    wk = nc.dram_tensor("wk", [HID, KC], BF16, kind="ExternalInput")
    wv = nc.dram_tensor("wv", [HID, KC], BF16, kind="ExternalInput")
    wo = nc.dram_tensor("wo", [QC, HOUT], BF16, kind="ExternalInput")
    cosb = nc.dram_tensor("cosb", [128, T], BF16, kind="ExternalInput")
    sinb = nc.dram_tensor("sinb", [128, T], BF16, kind="ExternalInput")  # sign-folded
    maskb = nc.dram_tensor("maskb", [128, MW], BF16, kind="ExternalInput")
    outp = nc.dram_tensor("outp", [HOUT, T], F32, kind="ExternalOutput")

    inv_sqrt_d = 1.0 / math.sqrt(D)

    with tile.TileContext(nc) as tc:
        with (
            tc.tile_pool(name="resident", bufs=1) as res,
            tc.tile_pool(name="const", bufs=1) as const,
        ):
            # resident SBUF arrays (live across all phases)
            qt = [res.tile([128, T], BF16, tag=f"qt{h}", name=f"qt{h}") for h in range(NQL)]
            kt = [res.tile([128, T], BF16, tag=f"kt{h}", name=f"kt{h}") for h in range(NKVL)]
            vt = [res.tile([128, KC], BF16, tag=f"v{t}", name=f"v{t}") for t in range(NT128)]
            cos_sb = res.tile([128, T], BF16, tag="cos", name="cos_sb")
            sin_sb = res.tile([128, T], BF16, tag="sin", name="sin_sb")
            mask_sb = res.tile([128, MW], BF16, tag="mask", name="mask_sb")
            ones_col = const.tile([128, 1], BF16)
            ones_row = const.tile([1, 128], BF16)
            nc.vector.memset(ones_col[:], 1.0)
            nc.vector.memset(ones_row[:], 1.0)

            # ---------------- phase 1: projections ----------------
            # XT resident in SBUF for the whole phase: fetched from HBM once.
            with (
                tc.tile_pool(name="xts", bufs=1) as xpool,
                tc.tile_pool(name="wt", bufs=2) as wpool,
                tc.tile_pool(name="rope_tmp", bufs=2) as rpool,
                tc.tile_pool(name="pj_psum", bufs=1, space="PSUM") as pp,
            ):
                xtsb = [xpool.tile([128, T], BF16, tag=f"x{k}", name=f"x{k}")
                        for k in range(KX)]
                for k in range(KX):
                    nc.sync.dma_start(out=xtsb[k][:],
                                      in_=xt[k * 128:(k + 1) * 128, :])
                nc.sync.dma_start(out=cos_sb[:], in_=cosb[:])
                nc.sync.dma_start(out=sin_sb[:], in_=sinb[:])
                nc.sync.dma_start(out=mask_sb[:], in_=maskb[:])

                def rope_evict(ps, dst_ap, tok0):
                    """dst = ps*cos + rot_half(ps)*sin  (sin sign-folded)."""
                    cw = cos_sb[:, tok0:tok0 + TQ]
                    sw = sin_sb[:, tok0:tok0 + TQ]
                    r = rpool.tile([128, TQ], F32, tag="rot", name="rot")
                    nc.scalar.copy(r[0:64, :], ps[64:128, :])
                    nc.scalar.copy(r[64:128, :], ps[0:64, :])
                    t1 = rpool.tile([128, TQ], F32, tag="t1", name="t1")
                    nc.vector.tensor_tensor(t1[:], ps[:], cw, op=AluOpType.mult)
                    nc.vector.tensor_tensor(r[:], r[:], sw, op=AluOpType.mult)
                    nc.vector.tensor_tensor(dst_ap, t1[:], r[:], op=AluOpType.add)

                # Jobs: (wsrc, dst, h0, nheads, vgroup).  K job first, then Q
                # head-pairs, then Q singles each fused with a V quarter-sweep
                # (V ldweights hide under the Q matmul streams).
                VG = 4
                jobs = [(wk, kt, 0, 2, None),
                        (wq, qt, 0, 2, None),
                        (wq, qt, 2, 2, None)]
                for i in range(4):
                    jobs.append((wq, qt, 4 + i, 1, i * VG))

                for wsrc, dst, h0, nheads, vg0 in jobs:
                    ps = [[pp.tile([128, TQ], F32, tag=f"pp{i * NTOK + j}",
                                   name=f"pj{i}{j}")
                           for j in range(NTOK)] for i in range(nheads)]
                    if vg0 is not None:
                        psv = [pp.tile([128, TQ], F32, tag=f"pp{4 + i}",
                                       name=f"pv{i}")
                               for i in range(VG)]
                    for k in range(KX):
                        wt_sb = wpool.tile([128, nheads * 128], BF16, tag="w", name="wt_sb")
                        nc.sync.dma_start(
                            out=wt_sb[:],
                            in_=wsrc[k * 128:(k + 1) * 128,
                                     h0 * 128:(h0 + nheads) * 128])
                        if vg0 is not None:
                            wv_sb = wpool.tile([128, KC], BF16, tag="wv", name="wv_sb")
                            nc.sync.dma_start(out=wv_sb[:],
                                              in_=wv[k * 128:(k + 1) * 128, :])
                        for i in range(nheads):
                            for j in range(NTOK):
                                nc.tensor.matmul(
                                    ps[i][j][:],
                                    lhsT=wt_sb[:, i * 128:(i + 1) * 128],
                                    rhs=xtsb[k][:, j * TQ:(j + 1) * TQ],
                                    start=(k == 0), stop=(k == KX - 1))
                        if vg0 is not None:
                            for i in range(VG):
                                t0 = (vg0 + i) * 128
                                nc.tensor.matmul(
                                    psv[i][:, 0:KC],
                                    lhsT=xtsb[k][:, t0:t0 + 128],
                                    rhs=wv_sb[:],
                                    start=(k == 0), stop=(k == KX - 1))
                    for i in range(nheads):
                        for j in range(NTOK):
                            rope_evict(ps[i][j], dst[h0 + i][:, j * TQ:(j + 1) * TQ],
                                       j * TQ)
                    if vg0 is not None:
                        for i in range(VG):
                            nc.vector.tensor_copy(vt[vg0 + i][:], psv[i][:, 0:KC])

            # ---------------- phases 2+3 ----------------
            with tc.tile_pool(name="post", bufs=1) as post:
                ot = [post.tile([128, T], BF16, tag=f"ot{h}", name=f"ot{h}")
                      for h in range(NQL)]

                # phase 2: attention, software-pipelined over (qi, h) jobs.
                # Job N's S^T matmuls + mask + exp + DVE row-sum accumulation
                # interleave with job N-1's AV matmuls; one ones-col matmul
                # per job turns the accumulated exp-sum into row sums, and the
                # 1/sum broadcast-normalize runs off the PE's critical path.
                with (
                    tc.tile_pool(name="es", bufs=2) as epool,
                    tc.tile_pool(name="at_small", bufs=2) as spool,
                    tc.tile_pool(name="ps_s", bufs=3, space="PSUM") as psum_s,
                    tc.tile_pool(name="ps_o", bufs=2, space="PSUM") as psum_o,
                    tc.tile_pool(name="ps_r", bufs=1, space="PSUM") as psum_r,
                    tc.tile_pool(name="ps_b", bufs=1, space="PSUM") as psum_b,
                ):
                    def emit_s_tile(job, ki):
                        ps_s = psum_s.tile([128, TQ], F32, tag="s", name="ps_s")
                        nc.tensor.matmul(
                            ps_s[:],
                            lhsT=kt[job["kvh"]][:, ki * 128:(ki + 1) * 128],
                            rhs=qt[job["h"]][:, job["q0"]:job["q0"] + TQ],
                            start=True, stop=True)
                        if ki >= job["nk"] - NKT:  # diagonal band: add mask
                            off = ki * 128 - job["q0"]
                            mv = mask_sb[:, (TQ - 128) - off:
                                         (TQ - 128) - off + TQ]
                            nc.vector.tensor_tensor(ps_s[:], ps_s[:], mv,
                                                    op=AluOpType.add)
                        e = epool.tile([128, TQ], BF16, tag=f"e{ki}",
                                       name=f"e{ki}")
                        nc.scalar.activation(e[:], ps_s[:], AF.Exp,
                                             scale=inv_sqrt_d)
                        job["es"].append(e)

                    def emit_av_tile(job, ki):
                        if ki == 0:
                            job["ps_o"] = psum_o.tile([128, TQ], F32, tag="o", name="ps_o")
                        nc.tensor.matmul(
                            job["ps_o"][:],
                            lhsT=vt[ki][:, job["kvh"] * D:(job["kvh"] + 1) * D],
                            rhs=job["es"][ki][:],
                            start=(ki == 0), stop=(ki == job["nk"] - 1))

                    def emit_sums(job):
                        # row sums of exp: consecutive ones-col matmuls so the
                        # stationary operand is loaded once for the whole chain
                        ps_row = psum_r.tile([1, TQ], F32, tag="row", name="ps_row")
                        for ki in range(job["nk"]):
                            nc.tensor.matmul(ps_row[:], lhsT=ones_col[:],
                                             rhs=job["es"][ki][:],
                                             start=(ki == 0),
                                             stop=(ki == job["nk"] - 1))
                        job["ps_row"] = ps_row

                    def emit_norm(job):
                        rsb = spool.tile([1, TQ], BF16, tag="rsb", name="rsb")
                        nc.vector.tensor_copy(rsb[:], job["ps_row"][:])
                        ps_bc = psum_b.tile([128, TQ], F32, tag="b", name="ps_bc")
                        nc.tensor.matmul(ps_bc[:], lhsT=ones_row[:],
                                         rhs=rsb[:], start=True, stop=True)
                        bcr = spool.tile([128, TQ], F32, tag="bcr", name="bcr")
                        nc.vector.reciprocal(bcr[:], ps_bc[:])
                        nc.vector.tensor_tensor(
                            ot[job["h"]][:, job["q0"]:job["q0"] + TQ],
                            job["ps_o"][:], bcr[:], op=AluOpType.mult)

                    prev = None
                    for qi in range(NTOK):
                        for h in range(NQL):
                            cur = {"qi": qi, "h": h, "kvh": h // GRP,
                                   "nk": (qi + 1) * NKT, "q0": qi * TQ,
                                   "es": []}
                            nk_p = prev["nk"] if prev else 0
                            for ki in range(max(cur["nk"], nk_p)):
                                if ki < cur["nk"]:
                                    emit_s_tile(cur, ki)
                                if prev is not None and ki < nk_p:
                                    emit_av_tile(prev, ki)
                            if prev is not None:
                                emit_sums(prev)
                                emit_norm(prev)
                            prev = cur
                    for ki in range(prev["nk"]):
                        emit_av_tile(prev, ki)
                    emit_sums(prev)
                    emit_norm(prev)

                # ---------------- phase 3: output projection ----------------
                CT = QC // 128  # contraction chunks (== NQL)
                with (
                    tc.tile_pool(name="wo_sb", bufs=2) as wopool,
                    tc.tile_pool(name="ob", bufs=4) as obpool,
                    tc.tile_pool(name="po_psum", bufs=2, space="PSUM") as pop,
                ):
                    NG = 4  # n-tiles per weight fetch group
                    for ng in range(0, HOUT // 128, NG):
                        gn = min(NG, HOUT // 128 - ng)
                        wos = []
                        for c in range(CT):
                            w = wopool.tile([128, gn * 128], BF16,
                                            tag=f"wo{c}", name=f"wosb{c}")
                            nc.sync.dma_start(
                                out=w[:], in_=wo[c * 128:(c + 1) * 128,
                                                ng * 128:(ng + gn) * 128])
                            wos.append(w)
                        for i in range(gn):
                            ni = ng + i
                            ps = [pop.tile([128, TQ], F32, tag=f"po{j}",
                                           name=f"po{j}")
                                  for j in range(NTOK)]
                            for c in range(CT):
                                for j in range(NTOK):
                                    nc.tensor.matmul(
                                        ps[j][:],
                                        lhsT=wos[c][:, i * 128:(i + 1) * 128],
                                        rhs=ot[c][:, j * TQ:(j + 1) * TQ],
                                        start=(c == 0), stop=(c == CT - 1))
                            for j in range(NTOK):
                                ob = obpool.tile([128, TQ], F32, tag="ob", name="ob")
                                nc.scalar.copy(ob[:], ps[j][:])
                                nc.sync.dma_start(
                                    out=outp[ni * 128:(ni + 1) * 128,
                                             j * TQ:(j + 1) * TQ],
                                    in_=ob[:])
    legalize_wait_counts(nc)
    return nc


def legalize_wait_counts(nc):
    """walrus DIRECT2D descriptors accept a single sync-wait; Tile can emit
    more (data wait + queue-head wait).  Hoist excess waits onto
    EventSemaphore instructions inserted just before, on the same engine."""
    n_new = 0
    for f in nc.m.functions:
        for blk in f.blocks:
            idx = 0
            insts = blk.instructions
            while idx < len(insts):
                inst = insts[idx]
                si = getattr(inst, "sync_info", None)
                cap = 2 if isinstance(inst, mybir.InstEventSemaphore) else 1
                waits = list(si.on_wait) if si is not None and si.on_wait else []
                if len(waits) > cap:
                    keep, extra = waits[-cap:], waits[:-cap]
                    si.on_wait = keep
                    for i in range(0, len(extra), 2):
                        ev = mybir.InstEventSemaphore(
                            name=f"waitsplit_{n_new}", ins=[], outs=[])
                        n_new += 1
                        ev.engine = inst.engine
                        ev.sync_info = mybir.SyncInfo(
                            on_wait=extra[i:i + 2], on_update=[])
                        nc.register_instruction(ev)
                        insts.insert(idx, ev)
                        idx += 1
                idx += 1
    return n_new


def _host_inputs(hidden_states, position_ids, Wq, Wk, Wv, Wo):
    """Build the 8 per-core input maps."""
    hs = np.asarray(hidden_states, dtype=np.float32)
    pos = np.asarray(position_ids)
    Wq = np.asarray(Wq, dtype=np.float32)
    Wk = np.asarray(Wk, dtype=np.float32)
    Wv = np.asarray(Wv, dtype=np.float32)
    Wo = np.asarray(Wo, dtype=np.float32)
    b, s, h = hs.shape
    qc = h // TP
    kc = (NUM_KV_HEADS * D) // TP
    bf = ml_dtypes.bfloat16

    # rope tables per batch, feature-major, sin sign-folded for rotate_half
    inv_freq = 1.0 / (ROPE_THETA ** (np.arange(0, D, 2, dtype=np.float32) / D))
    maps = []
    TQ = 512
    mw = TQ + (TQ - 128)
    i_idx = np.arange(128)[:, None]
    m_idx = np.arange(mw)[None, :]
    maskb = np.where(m_idx >= i_idx + (TQ - 128), 0.0, MASK_VAL).astype(bf)

    for c in range(DP * TP):
        bb, r = c // TP, c % TP
        t = pos[bb].astype(np.float64)  # [s]
        ang = t[None, :] * np.concatenate([inv_freq, inv_freq])[:, None]  # [128, s]
        cosb = np.cos(ang).astype(bf)
        sinb = np.sin(ang)
        sinb[0:64, :] *= -1.0  # rotate_half sign fold
        sinb = sinb.astype(bf)
        maps.append({
            "xt": np.ascontiguousarray(hs[bb].T).astype(bf),
            "wq": np.ascontiguousarray(Wq[:, r * qc:(r + 1) * qc]).astype(bf),
            "wk": np.ascontiguousarray(Wk[:, r * kc:(r + 1) * kc]).astype(bf),
            "wv": np.ascontiguousarray(Wv[:, r * kc:(r + 1) * kc]).astype(bf),
            "wo": np.ascontiguousarray(Wo[r * qc:(r + 1) * qc, :]).astype(bf),
            "cosb": cosb,
            "sinb": sinb,
            "maskb": maskb,
        })
    return maps


_NC_CACHE = {}


def _get_nc():
    if "nc" not in _NC_CACHE:
        _NC_CACHE["nc"] = build_nc()
    return _NC_CACHE["nc"]


def kernel(hidden_states, position_ids, Wq, Wk, Wv, Wo, _results_hook=None):
    from concourse.bass_utils import run_bass_kernel_spmd

    maps = _host_inputs(hidden_states, position_ids, Wq, Wk, Wv, Wo)
    nc = _get_nc()
    res = run_bass_kernel_spmd(nc, maps, list(range(DP * TP)))
    if _results_hook is not None:
        _results_hook(res)
    b, s, h = np.asarray(hidden_states).shape
    out = np.zeros((b, s, h), dtype=np.float32)
    for c in range(DP * TP):
        bb = c // TP
        out[bb] += res.results[c]["outp"].T
    return out


if __name__ == "__main__":
    # smoke: build the full-size program and print instruction counts
    nc = build_nc()
    print("built ok")


# revision 5
# speedup vs baseline: 1.3434x; 1.2914x over previous
"""Llama GQA attention (b=2, s=2048, h=4096, 32 Q heads / 8 KV heads, rope)
as a Bass/Tile kernel for 8 Trainium2 NeuronCores.

Sharding: data-parallel over batch (2) x tensor-parallel over heads (4).
Core c = (b, r), b = c // 4, r = c % 4 handles batch b with Q heads
[8r, 8r+8) and KV heads [2r, 2r+2).  Wq/Wk/Wv column-sharded, Wo
row-sharded; per-core output is a partial sum over the TP group which the
host reduces (fp32 adds).

On-core dataflow (all activations feature-major, i.e. transposed):
  XT [H, T] is DMA'd into SBUF ONCE and stays resident through phase 1;
  all projection matmuls stream it from SBUF (no HBM refetch).
  QT/KT [heads*128, T] with RoPE fused at PSUM eviction; V [T, 256]
  token-major, its transposing matmuls fused into the single-Q-head jobs
  so their ldweights hide under the Q matmul streams.
  Attention runs as a software pipeline over (q-tile, head) jobs:
  S^T[k,q] = KT-tile.T @ QT (PE) + causal mask (DVE) + exp (ACT) of job N
  interleave with AV matmuls of job N-1, so ACT never gates the PE.
  Row sums via DVE accumulation of the exp tiles plus ONE ones-vector
  matmul per job (not per tile); normalization applied at O^T eviction
  through a PE outer-product broadcast of 1/sum.
  O^T tiles feed the Wo projection producing OUT^T [H, T] which the host
  transposes / reduces.
"""

import math
import sys

import numpy as np

for _p in ("/opt/trn_rl_repo",):
    if _p not in sys.path:
        sys.path.insert(0, _p)

import ml_dtypes  # noqa: E402

import concourse.bass as bass  # noqa: E402
import concourse.mybir as mybir  # noqa: E402
import concourse.tile as tile  # noqa: E402
from concourse.alu_op_type import AluOpType  # noqa: E402

F32 = mybir.dt.float32
BF16 = mybir.dt.bfloat16
AF = mybir.ActivationFunctionType

# full problem constants
B, S, H = 2, 2048, 4096
NUM_HEADS, NUM_KV_HEADS, D = 32, 8, 128
ROPE_THETA = 10000.0
TP, DP = 4, 2
MASK_VAL = -30000.0


def build_nc(T=S, HID=H, NQL=NUM_HEADS // TP, NKVL=NUM_KV_HEADS // TP,
             HOUT=H, TQ=512):
    """One-core SPMD program.  T tokens, HID hidden, NQL local Q heads,
    NKVL local KV heads, HOUT output features, TQ q-tile width."""
    assert T % TQ == 0 and TQ % 128 == 0 and HID % 128 == 0
    GRP = NQL // NKVL            # q heads per kv head
    QC = NQL * D                 # local q columns
    KC = NKVL * D                # local kv columns
    KX = HID // 128              # contraction chunks for projections
    NTOK = T // TQ               # token tiles of width TQ
    NT128 = T // 128             # token tiles of width 128
    NKT = TQ // 128              # 128-wide k tiles per q tile
    MW = TQ + (TQ - 128)         # additive causal mask width

    nc = bass.Bass()
    xt = nc.dram_tensor("xt", [HID, T], BF16, kind="ExternalInput")
    wq = nc.dram_tensor("wq", [HID, QC], BF16, kind="ExternalInput")
    wk = nc.dram_tensor("wk", [HID, KC], BF16, kind="ExternalInput")
    wv = nc.dram_tensor("wv", [HID, KC], BF16, kind="ExternalInput")
    wo = nc.dram_tensor("wo", [QC, HOUT], BF16, kind="ExternalInput")
    cosb = nc.dram_tensor("cosb", [128, T], BF16, kind="ExternalInput")
    sinb = nc.dram_tensor("sinb", [128, T], BF16, kind="ExternalInput")  # sign-folded
    maskb = nc.dram_tensor("maskb", [128, MW], BF16, kind="ExternalInput")
    outp = nc.dram_tensor("outp", [HOUT, T], F32, kind="ExternalOutput")

    inv_sqrt_d = 1.0 / math.sqrt(D)

    with tile.TileContext(nc) as tc:
        with (
            tc.tile_pool(name="resident", bufs=1) as res,
            tc.tile_pool(name="const", bufs=1) as const,
        ):
            # resident SBUF arrays (live across all phases)
            qt = [res.tile([128, T], BF16, tag=f"qt{h}", name=f"qt{h}") for h in range(NQL)]
            kt = [res.tile([128, T], BF16, tag=f"kt{h}", name=f"kt{h}") for h in range(NKVL)]
            vt = [res.tile([128, KC], BF16, tag=f"v{t}", name=f"v{t}") for t in range(NT128)]
            cos_sb = res.tile([128, T], BF16, tag="cos", name="cos_sb")
            sin_sb = res.tile([128, T], BF16, tag="sin", name="sin_sb")
            mask_sb = res.tile([128, MW], BF16, tag="mask", name="mask_sb")
            ones_col = const.tile([128, 1], BF16)
            ones_row = const.tile([1, 128], BF16)
            nc.vector.memset(ones_col[:], 1.0)
            nc.vector.memset(ones_row[:], 1.0)

            # ---------------- phase 1: projections ----------------
            # XT resident in SBUF for the whole phase: fetched from HBM once.
            with (
                tc.tile_pool(name="xts", bufs=1) as xpool,
                tc.tile_pool(name="wt", bufs=8) as wpool,
                tc.tile_pool(name="rope_tmp", bufs=2) as rpool,
                tc.tile_pool(name="pj_psum", bufs=1, space="PSUM") as pp,
            ):
                xtsb = [xpool.tile([128, T], BF16, tag=f"x{k}", name=f"x{k}")
                        for k in range(KX)]

                def rope_evict(ps, dst_ap, tok0):
                    """dst = ps*cos + rot_half(ps)*sin  (sin sign-folded)."""
                    cw = cos_sb[:, tok0:tok0 + TQ]
                    sw = sin_sb[:, tok0:tok0 + TQ]
                    r = rpool.tile([128, TQ], F32, tag="rot", name="rot")
                    nc.scalar.copy(r[0:64, :], ps[64:128, :])
                    nc.scalar.copy(r[64:128, :], ps[0:64, :])
                    t1 = rpool.tile([128, TQ], F32, tag="t1", name="t1")
                    nc.vector.tensor_tensor(t1[:], ps[:], cw, op=AluOpType.mult)
                    nc.vector.tensor_tensor(r[:], r[:], sw, op=AluOpType.mult)
                    nc.vector.tensor_tensor(dst_ap, t1[:], r[:], op=AluOpType.add)

                # Jobs: (wsrc, dst, h0, nheads, vgroup).  K job first, then Q
                # head-pairs, then Q singles each fused with a V quarter-sweep
                # (V ldweights hide under the Q matmul streams).
                VG = 4
                jobs = [(wk, kt, 0, 2, None),
                        (wq, qt, 0, 2, None),
                        (wq, qt, 2, 2, None)]
                for i in range(4):
                    jobs.append((wq, qt, 4 + i, 1, i * VG))

                for job_i, (wsrc, dst, h0, nheads, vg0) in enumerate(jobs):
                    ps = [[pp.tile([128, TQ], F32, tag=f"pp{i * NTOK + j}",
                                   name=f"pj{i}{j}")
                           for j in range(NTOK)] for i in range(nheads)]
                    if vg0 is not None:
                        psv = [pp.tile([128, TQ], F32, tag=f"pp{4 + i}",
                                       name=f"pv{i}")
                               for i in range(VG)]
                    for k in range(KX):
                        if job_i == 0:
                            nc.sync.dma_start(out=xtsb[k][:],
                                              in_=xt[k * 128:(k + 1) * 128, :])
                            if k == 8:
                                nc.sync.dma_start(out=cos_sb[:], in_=cosb[:])
                                nc.sync.dma_start(out=sin_sb[:], in_=sinb[:])
                            if k == 16:
                                nc.sync.dma_start(out=mask_sb[:], in_=maskb[:])
                        wt_sb = wpool.tile([128, nheads * 128], BF16, tag="w", name="wt_sb")
                        nc.sync.dma_start(
                            out=wt_sb[:],
                            in_=wsrc[k * 128:(k + 1) * 128,
                                     h0 * 128:(h0 + nheads) * 128])
                        if vg0 is not None:
                            wv_sb = wpool.tile([128, KC], BF16, tag="wv", name="wv_sb")
                            nc.sync.dma_start(out=wv_sb[:],
                                              in_=wv[k * 128:(k + 1) * 128, :])
                        for i in range(nheads):
                            for j in range(NTOK):
                                nc.tensor.matmul(
                                    ps[i][j][:],
                                    lhsT=wt_sb[:, i * 128:(i + 1) * 128],
                                    rhs=xtsb[k][:, j * TQ:(j + 1) * TQ],
                                    start=(k == 0), stop=(k == KX - 1))
                        if vg0 is not None:
                            for i in range(VG):
                                t0 = (vg0 + i) * 128
                                nc.tensor.matmul(
                                    psv[i][:, 0:KC],
                                    lhsT=xtsb[k][:, t0:t0 + 128],
                                    rhs=wv_sb[:],
                                    start=(k == 0), stop=(k == KX - 1))
                    for i in range(nheads):
                        for j in range(NTOK):
                            rope_evict(ps[i][j], dst[h0 + i][:, j * TQ:(j + 1) * TQ],
                                       j * TQ)
                    if vg0 is not None:
                        for i in range(VG):
                            nc.vector.tensor_copy(vt[vg0 + i][:], psv[i][:, 0:KC])

            # ---------------- phases 2+3 ----------------
            with tc.tile_pool(name="post", bufs=1) as post:
                ot = [post.tile([128, T], BF16, tag=f"ot{h}", name=f"ot{h}")
                      for h in range(NQL)]

                # phase 2: attention, software-pipelined over (qi, h) jobs.
                # Job N's S^T matmuls + mask + exp + DVE row-sum accumulation
                # interleave with job N-1's AV matmuls; one ones-col matmul
                # per job turns the accumulated exp-sum into row sums, and the
                # 1/sum broadcast-normalize runs off the PE's critical path.
                with (
                    tc.tile_pool(name="es", bufs=2) as epool,
                    tc.tile_pool(name="at_small", bufs=2) as spool,
                    tc.tile_pool(name="ps_s", bufs=3, space="PSUM") as psum_s,
                    tc.tile_pool(name="ps_o", bufs=2, space="PSUM") as psum_o,
                    tc.tile_pool(name="ps_r", bufs=1, space="PSUM") as psum_r,
                    tc.tile_pool(name="ps_b", bufs=1, space="PSUM") as psum_b,
                ):
                    def emit_s_tile(job, ki):
                        ps_s = psum_s.tile([128, TQ], F32, tag="s", name="ps_s")
                        nc.tensor.matmul(
                            ps_s[:],
                            lhsT=kt[job["kvh"]][:, ki * 128:(ki + 1) * 128],
                            rhs=qt[job["h"]][:, job["q0"]:job["q0"] + TQ],
                            start=True, stop=True)
                        if ki >= job["nk"] - NKT:  # diagonal band: add mask
                            off = ki * 128 - job["q0"]
                            mv = mask_sb[:, (TQ - 128) - off:
                                         (TQ - 128) - off + TQ]
                            nc.vector.tensor_tensor(ps_s[:], ps_s[:], mv,
                                                    op=AluOpType.add)
                        e = epool.tile([128, TQ], BF16, tag=f"e{ki}",
                                       name=f"e{ki}")
                        nc.scalar.activation(e[:], ps_s[:], AF.Exp,
                                             scale=inv_sqrt_d)
                        job["es"].append(e)

                    def emit_av_tile(job, ki):
                        if ki == 0:
                            job["ps_o"] = psum_o.tile([128, TQ], F32, tag="o", name="ps_o")
                        nc.tensor.matmul(
                            job["ps_o"][:],
                            lhsT=vt[ki][:, job["kvh"] * D:(job["kvh"] + 1) * D],
                            rhs=job["es"][ki][:],
                            start=(ki == 0), stop=(ki == job["nk"] - 1))

                    def emit_sums(job):
                        # row sums of exp: consecutive ones-col matmuls so the
                        # stationary operand is loaded once for the whole chain
                        ps_row = psum_r.tile([1, TQ], F32, tag="row", name="ps_row")
                        for ki in range(job["nk"]):
                            nc.tensor.matmul(ps_row[:], lhsT=ones_col[:],
                                             rhs=job["es"][ki][:],
                                             start=(ki == 0),
                                             stop=(ki == job["nk"] - 1))
                        job["ps_row"] = ps_row

                    def emit_norm(job):
                        rsb = spool.tile([1, TQ], BF16, tag="rsb", name="rsb")
                        nc.vector.tensor_copy(rsb[:], job["ps_row"][:])
                        ps_bc = psum_b.tile([128, TQ], F32, tag="b", name="ps_bc")
                        nc.tensor.matmul(ps_bc[:], lhsT=ones_row[:],
                                         rhs=rsb[:], start=True, stop=True)
                        bcr = spool.tile([128, TQ], F32, tag="bcr", name="bcr")
                        nc.vector.reciprocal(bcr[:], ps_bc[:])
                        nc.vector.tensor_tensor(
                            ot[job["h"]][:, job["q0"]:job["q0"] + TQ],
                            job["ps_o"][:], bcr[:], op=AluOpType.mult)

                    prev = None
                    for qi in range(NTOK):
                        for h in range(NQL):
                            cur = {"qi": qi, "h": h, "kvh": h // GRP,
                                   "nk": (qi + 1) * NKT, "q0": qi * TQ,
                                   "es": []}
                            nk_p = prev["nk"] if prev else 0
                            for ki in range(max(cur["nk"], nk_p)):
                                if ki < cur["nk"]:
                                    emit_s_tile(cur, ki)
                                if prev is not None and ki < nk_p:
                                    emit_av_tile(prev, ki)
                            if prev is not None:
                                emit_sums(prev)
                                emit_norm(prev)
                            prev = cur
                    for ki in range(prev["nk"]):
                        emit_av_tile(prev, ki)
                    emit_sums(prev)
                    emit_norm(prev)

                # ---------------- phase 3: output projection ----------------
                CT = QC // 128  # contraction chunks (== NQL)
                with (
                    tc.tile_pool(name="wo_sb", bufs=2) as wopool,
                    tc.tile_pool(name="ob", bufs=4) as obpool,
                    tc.tile_pool(name="po_psum", bufs=2, space="PSUM") as pop,
                ):
                    NG = 4  # n-tiles per weight fetch group
                    for ng in range(0, HOUT // 128, NG):
                        gn = min(NG, HOUT // 128 - ng)
                        wos = []
                        for c in range(CT):
                            w = wopool.tile([128, gn * 128], BF16,
                                            tag=f"wo{c}", name=f"wosb{c}")
                            nc.sync.dma_start(
                                out=w[:], in_=wo[c * 128:(c + 1) * 128,
                                                ng * 128:(ng + gn) * 128])
                            wos.append(w)
                        for i in range(gn):
                            ni = ng + i
                            ps = [pop.tile([128, TQ], F32, tag=f"po{j}",
                                           name=f"po{j}")
                                  for j in range(NTOK)]
                            for c in range(CT):
                                for j in range(NTOK):
                                    nc.tensor.matmul(
                                        ps[j][:],
                                        lhsT=wos[c][:, i * 128:(i + 1) * 128],
                                        rhs=ot[c][:, j * TQ:(j + 1) * TQ],
                                        start=(c == 0), stop=(c == CT - 1))
                            for j in range(NTOK):
                                ob = obpool.tile([128, TQ], F32, tag="ob", name="ob")
                                nc.scalar.copy(ob[:], ps[j][:])
                                nc.sync.dma_start(
                                    out=outp[ni * 128:(ni + 1) * 128,
                                             j * TQ:(j + 1) * TQ],
                                    in_=ob[:])
    legalize_wait_counts(nc)
    return nc


def legalize_wait_counts(nc):
    """walrus DIRECT2D descriptors accept a single sync-wait; Tile can emit
    more (data wait + queue-head wait).  Hoist excess waits onto
    EventSemaphore instructions inserted just before, on the same engine."""
    n_new = 0
    for f in nc.m.functions:
        for blk in f.blocks:
            idx = 0
            insts = blk.instructions
            while idx < len(insts):
                inst = insts[idx]
                si = getattr(inst, "sync_info", None)
                cap = 2 if isinstance(inst, mybir.InstEventSemaphore) else 1
                waits = list(si.on_wait) if si is not None and si.on_wait else []
                if len(waits) > cap:
                    keep, extra = waits[-cap:], waits[:-cap]
                    si.on_wait = keep
                    for i in range(0, len(extra), 2):
                        ev = mybir.InstEventSemaphore(
                            name=f"waitsplit_{n_new}", ins=[], outs=[])
                        n_new += 1
                        ev.engine = inst.engine
                        ev.sync_info = mybir.SyncInfo(
                            on_wait=extra[i:i + 2], on_update=[])
                        nc.register_instruction(ev)
                        insts.insert(idx, ev)
                        idx += 1
                idx += 1
    return n_new


def _host_inputs(hidden_states, position_ids, Wq, Wk, Wv, Wo):
    """Build the 8 per-core input maps."""
    hs = np.asarray(hidden_states, dtype=np.float32)
    pos = np.asarray(position_ids)
    Wq = np.asarray(Wq, dtype=np.float32)
    Wk = np.asarray(Wk, dtype=np.float32)
    Wv = np.asarray(Wv, dtype=np.float32)
    Wo = np.asarray(Wo, dtype=np.float32)
    b, s, h = hs.shape
    qc = h // TP
    kc = (NUM_KV_HEADS * D) // TP
    bf = ml_dtypes.bfloat16

    # rope tables per batch, feature-major, sin sign-folded for rotate_half
    inv_freq = 1.0 / (ROPE_THETA ** (np.arange(0, D, 2, dtype=np.float32) / D))
    maps = []
    TQ = 512
    mw = TQ + (TQ - 128)
    i_idx = np.arange(128)[:, None]
    m_idx = np.arange(mw)[None, :]
    maskb = np.where(m_idx >= i_idx + (TQ - 128), 0.0, MASK_VAL).astype(bf)

    for c in range(DP * TP):
        bb, r = c // TP, c % TP
        t = pos[bb].astype(np.float64)  # [s]
        ang = t[None, :] * np.concatenate([inv_freq, inv_freq])[:, None]  # [128, s]
        cosb = np.cos(ang).astype(bf)
        sinb = np.sin(ang)
        sinb[0:64, :] *= -1.0  # rotate_half sign fold
        sinb = sinb.astype(bf)
        maps.append({
            "xt": np.ascontiguousarray(hs[bb].T).astype(bf),
            "wq": np.ascontiguousarray(Wq[:, r * qc:(r + 1) * qc]).astype(bf),
            "wk": np.ascontiguousarray(Wk[:, r * kc:(r + 1) * kc]).astype(bf),
            "wv": np.ascontiguousarray(Wv[:, r * kc:(r + 1) * kc]).astype(bf),
            "wo": np.ascontiguousarray(Wo[r * qc:(r + 1) * qc, :]).astype(bf),
            "cosb": cosb,
            "sinb": sinb,
            "maskb": maskb,
        })
    return maps


_NC_CACHE = {}


def _get_nc():
    if "nc" not in _NC_CACHE:
        _NC_CACHE["nc"] = build_nc()
    return _NC_CACHE["nc"]


def kernel(hidden_states, position_ids, Wq, Wk, Wv, Wo, _results_hook=None):
    from concourse.bass_utils import run_bass_kernel_spmd

    maps = _host_inputs(hidden_states, position_ids, Wq, Wk, Wv, Wo)
    nc = _get_nc()
    res = run_bass_kernel_spmd(nc, maps, list(range(DP * TP)))
    if _results_hook is not None:
        _results_hook(res)
    b, s, h = np.asarray(hidden_states).shape
    out = np.zeros((b, s, h), dtype=np.float32)
    for c in range(DP * TP):
        bb = c // TP
        out[bb] += res.results[c]["outp"].T
    return out


if __name__ == "__main__":
    # smoke: build the full-size program and print instruction counts
    nc = build_nc()
    print("built ok")
